# revision 1
# baseline (speedup 1.0000x reference)
"""CP-factorized embedding lookup on 8 TRN2 NeuronCores.

Reference computes full[a,b,c,d,e,f] = sum_r U0[a,r]*...*U5[f,r], reshapes to a
(50000, 512) table, and gathers rows by x. We never materialize the table:

  out[n, e] = sum_r (U0[a_n,r]*U1[b_n,r]*U2[c_n,r]) * (U3[d,r]*U4[e2,r]*U5[f,r])
            = sum_r V[n, r] * W[e, r]

with v = 1000a + 25b + c and e = 64d + 8e2 + f.

Per core (1024 indices, data-parallel over the 8192 total), in two pipelined
512-index halves:
  1. broadcast x across 115 partitions (50+40+25 stacked factor rows) and
     decompose it in place with per-partition constants in a short 16-bit
     DVE chain (4x perf mode):
       rows  0:50  -> a      = floor(v/1000)   (1000 when v == 0: see below)
       rows 50:90  -> b + 50 = floor(v/25) - 40*floor(v/1000) + 50
       rows 90:115 -> c + 90 = (v-25000) - 25*(floor(v/25)-1000) + 90
     floor(v/d) = f32->i16 cast of (v + bias)*(1/d); the HW cast rounds to
     nearest even, bias = -(d/2 - 0.5) puts the value mid-interval, so the
     result is exact. Block 2 is offset by -25000 to fit int16. The
     padding mask is folded in: rows 0:50 use s2 = min(v, 1) and
     diff = a - 1000*s2 + 1000, which equals a for v > 0 and 1000 (no
     one-hot hit -> zero row) for v == 0.
  2. one-hot[115, 512] = is_equal(diff, iota); gather via one PE matmul
     with block-diag stacked [U0;U1;U2] as lhsT -> psum[96, 512];
     V = elementwise product of the three 32-row blocks
  3. W[32, 512] = Khatri-Rao of U3,U4,U5 built with two broadcast multiplies
     (U3/U4/U5 transposed on-chip through the PE)
  4. out chunk c: matmul(lhsT=V[:,128j:128j+128], rhs=W) -> psum, two chunks
     batched per [128, 1024] psum pair, one Scalar-engine copy -> SBUF,
     one DMA per 256 output rows

All small constant operands (decomposition table, iota, identity, stacked
U3..U5, block-diagonal [U0;U1;U2]) are packed host-side into one aux input
(pure rearrangement/zero-padding -- all arithmetic stays on device) so the
front end costs a single small DMA. Matmul operands are produced as float32r
(tf32-like, 1 row/cycle vs 4 for float32); one-hot entries are exact in any
dtype and the factor rounding error is ~1e-4 relative, far inside tolerance.
"""

import numpy as np

import concourse.bass as bass
import concourse.mybir as mybir
import concourse.tile as tile
from concourse import bacc
from concourse.bass_utils import run_bass_kernel_spmd

F32 = mybir.dt.float32
F32R = mybir.dt.float32r
I32 = mybir.dt.int32
I16 = mybir.dt.int16
U16 = mybir.dt.uint16
ALU = mybir.AluOpType

N_CORES = 8
PER_CORE = 1024           # indices per core (8192 / 8)
HALF = 512                # pipeline granularity (one PSUM bank of columns)
EMB = 512
RANK = 32
KV = 115                  # 50 + 40 + 25 stacked vocab-factor rows
MV = 96                   # 3 * RANK stacked outputs

R1000 = float(np.float32(1.0 / 1000.0))
R25 = float(np.float32(1.0 / 25.0))

# aux layout: [115, 7 + 24 + 32 + 96]
CC_OFF = 0      # [115, 7] decomposition constants + iota
ID_OFF = 7      # [24, 24] identity (rows 0:24)
U345_OFF = 31   # [24, 32] stacked U3;U4;U5 (rows 0:24)
UBLK_OFF = 63   # [115, 96] block-diag [U0;U1;U2]
ONES_OFF = 159  # [1, 115] row of ones (lhsT of the broadcast matmul)
AUX_W = 274

# matmul operand dtype: float32r streams 1 row/cycle (vs 4 for float32).
MM_DT = F32R


def _const_table() -> np.ndarray:
    """[115, 7] per-partition constants: b1, R1, b2, R2, K, OFF, iota.

    Chain (s1, s2 are f32->i16 floor stages; the cast rounds to nearest):
      s1 = rint((v + b1) * R1);  s2 = rint((v + b2) * R2)
      (rows 0:50 overwrite: s2 = min(v, 1))
      diff = s1 - (K*s2 - OFF)  ; onehot = (diff == iota)
    """
    cc = np.zeros((KV, 7), np.float32)
    rows = ((0, 50), (50, 90), (90, 115))
    vals = [
        # s1 = a; s2 = min(v,1); hit iff a == 1000*s2 - 1000 + p
        (-499.5, R1000, 0.0, 1.0, 1000.0, 1000.0),
        # s1 = q25; s2 = a; hit iff q25 == 40a - 50 + p  (p abs. row 50..89)
        (-12.0, R25, -499.5, R1000, 40.0, 50.0),
        # s1 = v-25000; s2 = q25-1000; hit iff s1 == 25*s2 - 90 + p
        (-25000.0, 1.0, -25012.0, R25, 25.0, 90.0),
    ]
    for (lo, hi), v6 in zip(rows, vals):
        cc[lo:hi, 0:6] = np.float32(v6)
    # OFF2 = OFF - row: tkp = K*s2 - OFF2 and the one-hot becomes a single
    # fused tensor_tensor is_equal(s1, tkp)
    cc[:, 5] -= np.arange(KV, dtype=np.float32)
    return cc


def _aux_table(us: list[np.ndarray]) -> np.ndarray:
    aux = np.zeros((KV, AUX_W), np.float32)
    aux[:, CC_OFF:CC_OFF + 7] = _const_table()
    aux[0:24, ID_OFF:ID_OFF + 24] = np.eye(24, dtype=np.float32)
    aux[0:8, U345_OFF:U345_OFF + 32] = us[3]
    aux[8:16, U345_OFF:U345_OFF + 32] = us[4]
    aux[16:24, U345_OFF:U345_OFF + 32] = us[5]
    aux[0:50, UBLK_OFF:UBLK_OFF + 32] = us[0]
    aux[50:90, UBLK_OFF + 32:UBLK_OFF + 64] = us[1]
    aux[90:115, UBLK_OFF + 64:UBLK_OFF + 96] = us[2]
    aux[0, ONES_OFF:ONES_OFF + KV] = 1.0
    return aux


def build():
    nc = bacc.Bacc("TRN2", target_bir_lowering=False, debug=False)

    x = nc.dram_tensor("x", [PER_CORE], I32, kind="ExternalInput")
    aux_d = nc.dram_tensor("aux", [KV, AUX_W], F32, kind="ExternalInput")
    out = nc.dram_tensor("out", [PER_CORE, EMB], F32, kind="ExternalOutput")

    NH = PER_CORE // HALF   # 2 halves
    NC2 = HALF // 256       # 2 two-chunk groups per half

    with tile.TileContext(nc) as tc:
        with (
            tc.tile_pool(name="const", bufs=1) as cpool,
            tc.tile_pool(name="work", bufs=2) as wpool,
            tc.tile_pool(name="vpsum", bufs=2, space="PSUM") as ppool,
            tc.tile_pool(name="osb", bufs=2) as opool,
            tc.tile_pool(name="opsum", bufs=2, space="PSUM") as oppool,
        ):
            # ---- broadcast x across the 115 stacked factor rows (one
            # full-width DMA on the sync ring); aux lands in parallel on
            # the scalar ring.
            aux = cpool.tile([KV, AUX_W], F32)
            nc.sync.dma_start(out=aux[:], in_=aux_d[:])
            xrep = cpool.tile([KV, PER_CORE], I32)
            nc.sync.dma_start(
                out=xrep[:], in_=x[:].unsqueeze(0).partition_broadcast(KV)
            )
            cc = aux[:, CC_OFF:CC_OFF + 7]
            idm = aux[0:24, ID_OFF:ID_OFF + 24]
            u345 = aux[0:24, U345_OFF:U345_OFF + 32]

            # f32r-rounded copy of the block-diag factors for the gather mm
            ublk = cpool.tile([KV, MV], MM_DT)
            nc.vector.tensor_copy(out=ublk[:], in_=aux[:, UBLK_OFF:UBLK_OFF + 96])

            # ---- W[r, e] = U3[d,r] * U4[e2,r] * U5[f,r],  e = 64d + 8e2 + f
            u345t_ps = ppool.tile([RANK, 24], F32, tag="pv")
            nc.tensor.transpose(u345t_ps[:], u345, idm)
            u345t = cpool.tile([RANK, 24], F32)
            nc.scalar.copy(out=u345t[:], in_=u345t_ps[:])
            t45 = cpool.tile([RANK, 64], F32)
            nc.vector.tensor_tensor(
                out=t45[:].rearrange("r (e f) -> r e f", e=8),
                in0=u345t[:, 8:16].unsqueeze(2).broadcast_to([RANK, 8, 8]),
                in1=u345t[:, 16:24].unsqueeze(1).broadcast_to([RANK, 8, 8]),
                op=ALU.mult,
            )
            wt = cpool.tile([RANK, EMB], MM_DT)
            nc.vector.tensor_tensor(
                out=wt[:].rearrange("r (d ef) -> r d ef", d=8),
                in0=u345t[:, 0:8].unsqueeze(2).broadcast_to([RANK, 8, 64]),
                in1=t45[:].unsqueeze(1).broadcast_to([RANK, 8, 64]),
                op=ALU.mult,
            )

            # ---- full-width 5-op decomposition chain straight off the
            # int32 broadcast (mixed int-in/f32-scalar tensor_scalar is
            # exact on HW: internal fp32 ALU + round-to-nearest int cast)
            s1 = cpool.tile([KV, PER_CORE], I16)
            nc.vector.tensor_scalar(
                out=s1[:], in0=xrep[:], scalar1=cc[:, 0:1], scalar2=cc[:, 1:2],
                op0=ALU.add, op1=ALU.mult,
            )
            s2 = cpool.tile([KV, PER_CORE], I16)
            nc.vector.tensor_scalar(
                out=s2[:], in0=xrep[:], scalar1=cc[:, 2:3], scalar2=cc[:, 3:4],
                op0=ALU.add, op1=ALU.mult,
            )
            # rows 0:50: s2 = min(v, 1) -> folds the v==0 padding mask into
            # the block-0 one-hot (no hit for v == 0 -> zero output row)
            nc.vector.tensor_scalar(
                out=s2[0:50, :], in0=xrep[0:50, :], scalar1=1.0, scalar2=1.0,
                op0=ALU.min, op1=ALU.mult,
            )
            tkp = cpool.tile([KV, PER_CORE], I16)
            nc.vector.tensor_scalar(
                out=tkp[:], in0=s2[:], scalar1=cc[:, 4:5], scalar2=cc[:, 5:6],
                op0=ALU.mult, op1=ALU.subtract,
            )
            onehot = cpool.tile([KV, PER_CORE], MM_DT)
            nc.vector.tensor_tensor(
                out=onehot[:], in0=s1[:], in1=tkp[:], op=ALU.is_equal
            )

            for h in range(NH):
                pv = ppool.tile([MV, HALF], F32, name=f"pv_{h}", tag="pv")
                nc.tensor.matmul(
                    pv[:], lhsT=ublk[:],
                    rhs=onehot[:, h * HALF:(h + 1) * HALF],
                    start=True, stop=True,
                )
                # DVE may read only one PSUM operand per op: stage block 0
                # to SBUF on the Scalar engine first.
                s0 = wpool.tile([RANK, HALF], F32, name=f"s0_{h}", tag="s0")
                nc.scalar.copy(out=s0[:], in_=pv[0:32, :])
                v01 = wpool.tile([RANK, HALF], F32, name=f"v01_{h}", tag="v01")
                nc.vector.tensor_tensor(
                    out=v01[:], in0=s0[:], in1=pv[32:64, :], op=ALU.mult
                )
                vth = cpool.tile([RANK, HALF], MM_DT, name=f"vt_{h}")
                nc.vector.tensor_tensor(
                    out=vth[:], in0=v01[:], in1=pv[64:96, :], op=ALU.mult
                )

                # two output chunks batched per [128, 1024] psum pair
                for g in range(NC2):
                    po2 = oppool.tile([128, 2 * EMB], F32, name=f"po_{h}{g}",
                                      tag="po")
                    for j in range(2):
                        nc.tensor.matmul(
                            po2[:, j * EMB:(j + 1) * EMB],
                            lhsT=vth[:, (2 * g + j) * 128:(2 * g + j + 1) * 128],
                            rhs=wt[:],
                            start=True, stop=True,
                        )
                    osb = opool.tile([128, 2 * EMB], F32, name=f"osb_{h}{g}",
                                     tag="osb")
                    if g == 0:
                        nc.scalar.copy(out=osb[:], in_=po2[:])
                    else:
                        nc.vector.tensor_copy(out=osb[:], in_=po2[:])
                    row0 = h * HALF + g * 256
                    nc.sync.dma_start(
                        out=out[row0:row0 + 256, :].rearrange(
                            "(j p) e -> p j e", p=128
                        ),
                        in_=osb[:].rearrange("p (j e) -> p j e", j=2),
                    )

    nc.compile()
    return nc


_CACHE: dict = {}


def _get_nc():
    if "nc" not in _CACHE:
        _CACHE["nc"] = build()
    return _CACHE["nc"]


def run(inputs, **spmd_kwargs):
    nc = _get_nc()
    x = np.ascontiguousarray(inputs["x"].reshape(-1), dtype=np.int32)
    us = [
        np.ascontiguousarray(inputs[f"U{j}"], dtype=np.float32) for j in range(6)
    ]
    aux = _aux_table(us)
    in_maps = []
    for i in range(N_CORES):
        in_maps.append({"x": x[i * PER_CORE:(i + 1) * PER_CORE], "aux": aux})
    res = run_bass_kernel_spmd(
        nc, in_maps, core_ids=list(range(N_CORES)), **spmd_kwargs
    )
    shards = [np.asarray(res.results[i]["out"]) for i in range(N_CORES)]
    full = np.concatenate(shards, axis=0).reshape(4, 2048, EMB)
    return full.astype(np.float32, copy=False), res


def kernel(**inputs) -> np.ndarray:
    return run(inputs)[0]



# revision 2
# speedup vs baseline: 1.0008x; 1.0008x over previous
"""CP-factorized embedding lookup on 8 TRN2 NeuronCores.

Reference computes full[a,b,c,d,e,f] = sum_r U0[a,r]*...*U5[f,r], reshapes to a
(50000, 512) table, and gathers rows by x. We never materialize the table:

  out[n, e] = sum_r (U0[a_n,r]*U1[b_n,r]*U2[c_n,r]) * (U3[d,r]*U4[e2,r]*U5[f,r])
            = sum_r V[n, r] * W[e, r]

with v = 1000a + 25b + c and e = 64d + 8e2 + f.

Per core (1024 indices, data-parallel over the 8192 total):
  1. x is shipped as uint16 (lossless: v < 50000 < 65536) and broadcast by one
     DMA across 115 partitions (50+40+25 stacked factor rows). The digit
     decomposition runs as a 4-op int16 DVE chain in 2x perf mode:
       s1  = rint((v + b1) * R1)          (f32->i16 cast rounds to nearest)
       s2  = rint((v + b2) * R2)
       tkp = K * s2 - OFF                 (per-partition constants)
       onehot = is_equal(s1, tkp) -> bf16
     Rows 0:50 compare a == p with the padding mask folded in as an affine
     step function: s2 = rint((v + 49999.95) * 1e-5) = (v >= 1), so v == 0
     hits no one-hot row -> zero output row. Rows 50:90 compare
     q25 == 40a - 50 + p, rows 90:115 compare (v-25000) == 25*(q25-1000)-90+p.
  2. gather via one PE matmul per 512-index half with block-diag stacked
     [U0;U1;U2] (bf16) as lhsT -> psum[96, 512]; V = elementwise product of
     the three 32-row blocks (scalar copy + two DVE mults).
  3. W[32, 512] = Khatri-Rao of U3,U4,U5 built with two broadcast multiplies
     on GpSimd from host-side-transposed factors (pure layout change).
  4. out chunk: matmul(lhsT=V[:,128c:+128], rhs=W[32,512] f32r) -> psum,
     two chunks per [128, 1024] psum pair, copied to SBUF as bf16 and DMA'd
     in sub-128KB pieces alternating both HWDGE rings (small final pieces
     shorten the drain tail). Host upcasts bf16 -> f32.
  A few warm-up matmuls on zero data keep the PE busy during the index chain
  so the output matmuls run at the ramped-up clock.

All constant operands (decomposition table, transposed U3..U5, block-diagonal
[U0;U1;U2]) are packed host-side into one aux input (pure rearrangement/
zero-padding -- all arithmetic stays on device).
"""

import numpy as np

import concourse.bass as bass
import concourse.mybir as mybir
import concourse.tile as tile
from concourse import bacc
from concourse.bass_utils import run_bass_kernel_spmd

F32 = mybir.dt.float32
F32R = mybir.dt.float32r
BF16 = mybir.dt.bfloat16
I16 = mybir.dt.int16
U16 = mybir.dt.uint16
ALU = mybir.AluOpType

N_CORES = 8
PER_CORE = 1024           # indices per core (8192 / 8)
HALF = 512                # pipeline granularity (one PSUM bank of columns)
EMB = 512
RANK = 32
KV = 115                  # 50 + 40 + 25 stacked vocab-factor rows
MV = 96                   # 3 * RANK stacked outputs

R1000 = float(np.float32(1.0 / 1000.0))
R25 = float(np.float32(1.0 / 25.0))

# aux layout: [115, 6 + 24 + 96]
CC_OFF = 0      # [115, 6] decomposition constants
U345_OFF = 6    # [32, 24] host-transposed U3;U4;U5 (rows 0:32)
UBLK_OFF = 30   # [115, 96] block-diag [U0;U1;U2]
AUX_W = 126

N_WARM = 6      # PE warm-up matmuls (ramp the tensor-engine p-state)


def _const_table() -> np.ndarray:
    """[115, 6] per-partition constants: b1, R1, b2, R2, K, OFF - row.

    s1 = rint((v+b1)*R1); s2 = rint((v+b2)*R2); hit iff s1 == K*s2 - OFF + p.
    """
    cc = np.zeros((KV, 6), np.float32)
    rows = ((0, 50), (50, 90), (90, 115))
    vals = [
        # s1 = a; s2 = (v >= 1); hit iff a == 1000*s2 - 1000 + p
        (-499.5, R1000, 49999.95, 1e-5, 1000.0, 1000.0),
        # s1 = q25; s2 = a; hit iff q25 == 40a - 50 + p  (p abs. row 50..89)
        (-12.0, R25, -499.5, R1000, 40.0, 50.0),
        # s1 = v-25000; s2 = q25-1000; hit iff s1 == 25*s2 - 90 + p
        (-25000.0, 1.0, -25012.0, R25, 25.0, 90.0),
    ]
    for (lo, hi), v6 in zip(rows, vals):
        cc[lo:hi, 0:6] = np.float32(v6)
    cc[:, 5] -= np.arange(KV, dtype=np.float32)
    return cc


def _aux_table(us: list[np.ndarray]) -> np.ndarray:
    aux = np.zeros((KV, AUX_W), np.float32)
    aux[:, CC_OFF:CC_OFF + 6] = _const_table()
    # host-side transpose (pure layout): u345t[r, j] = U{3,4,5}[j, r]
    aux[0:RANK, U345_OFF:U345_OFF + 8] = us[3].T
    aux[0:RANK, U345_OFF + 8:U345_OFF + 16] = us[4].T
    aux[0:RANK, U345_OFF + 16:U345_OFF + 24] = us[5].T
    aux[0:50, UBLK_OFF:UBLK_OFF + 32] = us[0]
    aux[50:90, UBLK_OFF + 32:UBLK_OFF + 64] = us[1]
    aux[90:115, UBLK_OFF + 64:UBLK_OFF + 96] = us[2]
    return aux


# output DMA pieces: (half, group, chunk, row_lo, row_hi, ring)
# whole 128-row chunks early; the last chunks split small to shorten the
# DMA drain tail (per-piece transfer time ~ bytes / 22.5 GB/s on one engine).
_OUT_PIECES = [
    (0, 0, 0, 0, 128, "sync"),
    (0, 0, 1, 0, 128, "scalar"),
    (0, 1, 0, 0, 128, "sync"),
    (0, 1, 1, 0, 128, "scalar"),
    (1, 0, 0, 0, 128, "sync"),
    (1, 0, 1, 0, 128, "scalar"),
    (1, 1, 0, 0, 64, "sync"),
    (1, 1, 0, 64, 128, "scalar"),
    (1, 1, 1, 0, 64, "sync"),
    (1, 1, 1, 64, 96, "scalar"),
    (1, 1, 1, 96, 128, "sync"),
]


def build():
    nc = bacc.Bacc("TRN2", target_bir_lowering=False, debug=False)

    x = nc.dram_tensor("x", [PER_CORE], U16, kind="ExternalInput")
    aux_d = nc.dram_tensor("aux", [KV, AUX_W], F32, kind="ExternalInput")
    out = nc.dram_tensor("out", [PER_CORE, EMB], BF16, kind="ExternalOutput")

    with tile.TileContext(nc) as tc:
        with (
            tc.tile_pool(name="const", bufs=1) as cpool,
            tc.tile_pool(name="work", bufs=2) as wpool,
            tc.tile_pool(name="vpsum", bufs=2, space="PSUM") as ppool,
            tc.tile_pool(name="osb", bufs=2) as opool,
            tc.tile_pool(name="opsum", bufs=2, space="PSUM") as oppool,
            tc.tile_pool(name="dpsum", bufs=1, space="PSUM") as dpool,
        ):
            # ---- input DMAs, one per HWDGE ring. The uint16 broadcast
            # (235KB) stripes over ~5 DMA engines; aux (58KB) rides scalar.
            xrep = cpool.tile([KV, PER_CORE], U16)
            nc.sync.dma_start(
                out=xrep[:], in_=x[:].unsqueeze(0).partition_broadcast(KV)
            )
            aux = cpool.tile([KV, AUX_W], F32)
            nc.scalar.dma_start(out=aux[:], in_=aux_d[:])
            cc = aux[:, CC_OFF:CC_OFF + 6]
            u345t = aux[0:RANK, U345_OFF:U345_OFF + 24]

            # bf16 copy of the block-diag factors for the gather matmul
            ublk = cpool.tile([KV, MV], BF16)
            nc.vector.tensor_copy(
                out=ublk[:], in_=aux[:, UBLK_OFF:UBLK_OFF + MV]
            )

            # ---- W[r, e] = U3[d,r] * U4[e2,r] * U5[f,r], e = 64d + 8e2 + f
            # (GpSimd; off the critical path while x is still in flight)
            t45 = cpool.tile([RANK, 64], F32)
            nc.gpsimd.tensor_tensor(
                out=t45[:].rearrange("r (e f) -> r e f", e=8),
                in0=u345t[:, 8:16].unsqueeze(2).broadcast_to([RANK, 8, 8]),
                in1=u345t[:, 16:24].unsqueeze(1).broadcast_to([RANK, 8, 8]),
                op=ALU.mult,
            )
            wt = cpool.tile([RANK, EMB], F32R)
            nc.gpsimd.tensor_tensor(
                out=wt[:].rearrange("r (d ef) -> r d ef", d=8),
                in0=u345t[:, 0:8].unsqueeze(2).broadcast_to([RANK, 8, 64]),
                in1=t45[:].unsqueeze(1).broadcast_to([RANK, 8, 64]),
                op=ALU.mult,
            )

            # ---- PE warm-up: zero-data matmuls sharing ublk as lhsT keep
            # the tensor engine continuously busy through the index chain so
            # the real matmuls run at the ramped clock.
            warm = cpool.tile([KV, HALF], BF16)
            nc.gpsimd.memset(warm[:], 0.0)
            pd = dpool.tile([MV, HALF], F32)
            for i in range(N_WARM):
                nc.tensor.matmul(
                    pd[:], lhsT=ublk[:], rhs=warm[:], start=True, stop=True
                )

            # ---- 4-op digit-decomposition chain, int16 2x DVE perf mode
            s1 = cpool.tile([KV, PER_CORE], I16)
            nc.vector.tensor_scalar(
                out=s1[:], in0=xrep[:], scalar1=cc[:, 0:1], scalar2=cc[:, 1:2],
                op0=ALU.add, op1=ALU.mult,
            )
            s2 = cpool.tile([KV, PER_CORE], I16)
            nc.vector.tensor_scalar(
                out=s2[:], in0=xrep[:], scalar1=cc[:, 2:3], scalar2=cc[:, 3:4],
                op0=ALU.add, op1=ALU.mult,
            )
            tkp = cpool.tile([KV, PER_CORE], I16)
            nc.vector.tensor_scalar(
                out=tkp[:], in0=s2[:], scalar1=cc[:, 4:5], scalar2=cc[:, 5:6],
                op0=ALU.mult, op1=ALU.subtract,
            )
            onehot = cpool.tile([KV, PER_CORE], BF16)
            nc.vector.tensor_tensor(
                out=onehot[:], in0=s1[:], in1=tkp[:], op=ALU.is_equal
            )

            osbs = {}
            for h in range(2):
                pv = ppool.tile([MV, HALF], F32, name=f"pv_{h}", tag="pv")
                nc.tensor.matmul(
                    pv[:], lhsT=ublk[:],
                    rhs=onehot[:, h * HALF:(h + 1) * HALF],
                    start=True, stop=True,
                )
                # DVE may read only one PSUM operand per op: stage block 0
                # to SBUF on the Scalar engine first.
                s0 = wpool.tile([RANK, HALF], F32, name=f"s0_{h}", tag="s0")
                nc.scalar.copy(out=s0[:], in_=pv[0:32, :])
                v01 = wpool.tile([RANK, HALF], F32, name=f"v01_{h}", tag="v01")
                nc.vector.tensor_tensor(
                    out=v01[:], in0=s0[:], in1=pv[32:64, :], op=ALU.mult
                )
                vth = cpool.tile([RANK, HALF], F32R, name=f"vt_{h}")
                nc.vector.tensor_tensor(
                    out=vth[:], in0=v01[:], in1=pv[64:96, :], op=ALU.mult
                )

                for g in range(2):
                    po2 = oppool.tile([128, 2 * EMB], F32, name=f"po_{h}{g}",
                                      tag="po")
                    for j in range(2):
                        nc.tensor.matmul(
                            po2[:, j * EMB:(j + 1) * EMB],
                            lhsT=vth[:, (2 * g + j) * 128:(2 * g + j + 1) * 128],
                            rhs=wt[:],
                            start=True, stop=True,
                        )
                    osb = opool.tile([128, 2 * EMB], BF16, name=f"osb_{h}{g}",
                                     tag="osb")
                    if g == 0:
                        nc.scalar.copy(out=osb[:], in_=po2[:])
                    else:
                        nc.vector.tensor_copy(out=osb[:], in_=po2[:])
                    osbs[(h, g)] = osb

                    # output pieces for this group
                    for (ph, pg, pj, lo, hi, ring) in _OUT_PIECES:
                        if ph != h or pg != g:
                            continue
                        r0 = h * HALF + g * 256 + pj * 128
                        eng = nc.sync if ring == "sync" else nc.scalar
                        eng.dma_start(
                            out=out[r0 + lo:r0 + hi, :],
                            in_=osb[lo:hi, pj * EMB:(pj + 1) * EMB],
                        )

    nc.compile()
    return nc


_CACHE: dict = {}


def _get_nc():
    if "nc" not in _CACHE:
        _CACHE["nc"] = build()
    return _CACHE["nc"]


def run(inputs, **spmd_kwargs):
    nc = _get_nc()
    x = np.ascontiguousarray(inputs["x"].reshape(-1)).astype(np.uint16)
    us = [
        np.ascontiguousarray(inputs[f"U{j}"], dtype=np.float32) for j in range(6)
    ]
    aux = _aux_table(us)
    in_maps = []
    for i in range(N_CORES):
        in_maps.append({"x": x[i * PER_CORE:(i + 1) * PER_CORE], "aux": aux})
    res = run_bass_kernel_spmd(
        nc, in_maps, core_ids=list(range(N_CORES)), **spmd_kwargs
    )
    shards = [
        np.asarray(res.results[i]["out"]).astype(np.float32)
        for i in range(N_CORES)
    ]
    full = np.concatenate(shards, axis=0).reshape(4, 2048, EMB)
    return full, res


def kernel(**inputs) -> np.ndarray:
    return run(inputs)[0]


# revision 9
# speedup vs baseline: 1.2589x; 1.2579x over previous
"""CP-factorized embedding lookup on 8 TRN2 NeuronCores.

Reference computes full[a,b,c,d,e,f] = sum_r U0[a,r]*...*U5[f,r], reshapes to a
(50000, 512) table, and gathers rows by x. We never materialize the table:

  out[n, e] = sum_r (U0[a_n,r]*U1[b_n,r]*U2[c_n,r]) * (U3[d,r]*U4[e2,r]*U5[f,r])
            = sum_r V[n, r] * W[e, r]

with v = 1000a + 25b + c and e = 64d + 8e2 + f.

Per core (1024 indices, data-parallel over the 8192 total):
  1. x ships as uint16 (lossless: v < 50000 < 65536) and is broadcast by one
     DMA across 115 partitions (50+40+25 stacked factor rows). The digit
     decomposition runs as a 4-op int16 DVE chain in 2x perf mode:
       s1  = rint((v + b1) * R1)          (f32->i16 cast rounds to nearest)
       s2  = rint((v + b2) * R2)
       tkp = K * s2 - OFF                 (per-partition constants)
       onehot = is_equal(s1, tkp) -> bf16 (split per 512-col half so the
                                           first gather matmul starts early)
     Rows 0:50 compare a == p with the padding mask folded in as an affine
     step function: s2 = rint((v + 49999.95) * 1e-5) = (v >= 1), so v == 0
     hits no one-hot row -> zero output row. Rows 50:90 compare
     q25 == 40a - 50 + p, rows 90:115 compare (v-25000) == 25*(q25-1000)-90+p.
  2. gather via one PE matmul per half with block-diag stacked [U0;U1;U2]
     (bf16, assembled on GpSimd from packed factors) -> psum[96, 512];
     V = product of the three 32-row blocks, computed in 256-col pieces
     (scalar copy + two DVE mults each) so output matmuls start sooner.
  3. W[32, 512] = Khatri-Rao of U3,U4,U5 via two broadcast multiplies on DVE
     from host-side-transposed factors, before the index chain arrives.
  4. out chunk: matmul(lhsT=V[:,128c:+128], rhs=W f32r) -> psum, two chunks
     per [128, 1024] psum pair, copied to SBUF as bf16 in two 512-col halves
     (Scalar || Vector) and DMA'd in sub-128KB pieces on both HWDGE rings
     with small final pieces to shorten the drain tail. Host upcasts -> f32.

Zero-data warm-up matmuls run on the PE from early startup through the index
chain so the tensor engine reaches its ramped clock (2x faster matmuls)
before the real gather/output matmuls issue.

All constant operands (decomposition table, transposed U3..U5, packed
U0/U1/U2) ride one aux input built host-side by pure rearrangement/zero-
padding -- all arithmetic stays on device.
"""

import numpy as np

import concourse.bass as bass
import concourse.mybir as mybir
import concourse.tile as tile
from concourse import bacc
from concourse.bass_utils import run_bass_kernel_spmd

F32 = mybir.dt.float32
F32R = mybir.dt.float32r
BF16 = mybir.dt.bfloat16
I16 = mybir.dt.int16
U16 = mybir.dt.uint16
ALU = mybir.AluOpType

N_CORES = 8
PER_CORE = 1024           # indices per core (8192 / 8)
HALF = 512                # pipeline granularity (one PSUM bank of columns)
EMB = 512
RANK = 32
KV = 115                  # 50 + 40 + 25 stacked vocab-factor rows
MV = 96                   # 3 * RANK stacked outputs

R1000 = float(np.float32(1.0 / 1000.0))
R25 = float(np.float32(1.0 / 25.0))

# aux layout: [115, 6 + 24] constants; aux2: [115, 96] block-diag factors
CC_OFF = 0      # [115, 6] decomposition constants
U345_OFF = 6    # [32, 24] host-transposed U3;U4;U5 (rows 0:32)
AUX_W = 30

N_WARM = 12     # PE warm-up matmuls before the gathers (p-state ramp)
N_FILL = 2      # PE gap fillers between gathers and output matmuls


def _const_table() -> np.ndarray:
    """[115, 6] per-partition constants: b1, R1, b2, R2, K, OFF - row.

    s1 = rint((v+b1)*R1); s2 = rint((v+b2)*R2); hit iff s1 == K*s2 - OFF + p.
    """
    cc = np.zeros((KV, 6), np.float32)
    rows = ((0, 50), (50, 90), (90, 115))
    vals = [
        # s1 = a; s2 = (v >= 1); hit iff a == 1000*s2 - 1000 + p
        (-499.5, R1000, 49999.95, 1e-5, 1000.0, 1000.0),
        # s1 = q25; s2 = a; hit iff q25 == 40a - 50 + p  (p abs. row 50..89)
        (-12.0, R25, -499.5, R1000, 40.0, 50.0),
        # s1 = v-25000; s2 = q25-1000; hit iff s1 == 25*s2 - 90 + p
        (-25000.0, 1.0, -25012.0, R25, 25.0, 90.0),
    ]
    for (lo, hi), v6 in zip(rows, vals):
        cc[lo:hi, 0:6] = np.float32(v6)
    cc[:, 5] -= np.arange(KV, dtype=np.float32)
    return cc


def _aux_table(us: list[np.ndarray]) -> tuple[np.ndarray, np.ndarray]:
    aux = np.zeros((KV, AUX_W), np.float32)
    aux[:, CC_OFF:CC_OFF + 6] = _const_table()
    # host-side transpose (pure layout): u345t[r, j] = U{3,4,5}[j, r]
    aux[0:RANK, U345_OFF:U345_OFF + 8] = us[3].T
    aux[0:RANK, U345_OFF + 8:U345_OFF + 16] = us[4].T
    aux[0:RANK, U345_OFF + 16:U345_OFF + 24] = us[5].T
    aux2 = np.zeros((KV, MV), np.float32)
    aux2[0:50, 0:32] = us[0]
    aux2[50:90, 32:64] = us[1]
    aux2[90:115, 64:96] = us[2]
    return aux, aux2


# output DMA pieces: (half, group, chunk, row_lo, row_hi, ring)
# whole 128-row chunks early; the last chunks split small to shorten the
# DMA drain tail (per-piece transfer time ~ bytes / 22.5 GB/s on one engine).
_OUT_PIECES = [
    (0, 0, 0, 0, 128, "sync"),
    (0, 0, 1, 0, 128, "scalar"),
    (0, 1, 0, 0, 128, "sync"),
    (0, 1, 1, 0, 128, "scalar"),
    (1, 0, 0, 0, 128, "sync"),
    (1, 0, 1, 0, 128, "scalar"),
    (1, 1, 0, 0, 64, "sync"),
    (1, 1, 0, 64, 128, "scalar"),
    (1, 1, 1, 0, 64, "sync"),
    (1, 1, 1, 64, 96, "scalar"),
    (1, 1, 1, 96, 128, "sync"),
]


def build():
    nc = bacc.Bacc("TRN2", target_bir_lowering=False, debug=False)

    x = nc.dram_tensor("x", [PER_CORE], U16, kind="ExternalInput")
    aux_d = nc.dram_tensor("aux", [KV, AUX_W], F32, kind="ExternalInput")
    aux2_d = nc.dram_tensor("aux2", [KV, MV], F32, kind="ExternalInput")
    out = nc.dram_tensor("out", [PER_CORE, EMB], BF16, kind="ExternalOutput")

    with tile.TileContext(nc) as tc:
        with (
            tc.tile_pool(name="const", bufs=1) as cpool,
            tc.tile_pool(name="work", bufs=2) as wpool,
            tc.tile_pool(name="vpsum", bufs=2, space="PSUM") as ppool,
            tc.tile_pool(name="osb", bufs=2) as opool,
            tc.tile_pool(name="opsum", bufs=2, space="PSUM") as oppool,
            tc.tile_pool(name="dpsum", bufs=1, space="PSUM") as dpool,
        ):
            # ---- input DMAs, one per HWDGE ring. The uint16 broadcast
            # (235KB) stripes over ~5 DMA engines; aux (28KB) rides scalar.
            xrep = cpool.tile([KV, PER_CORE], U16)
            nc.sync.dma_start(
                out=xrep[:], in_=x[:].unsqueeze(0).partition_broadcast(KV)
            )
            aux = cpool.tile([KV, AUX_W], F32)
            nc.scalar.dma_start(out=aux[:], in_=aux_d[:])
            aux2 = cpool.tile([KV, MV], F32)
            nc.scalar.dma_start(out=aux2[:], in_=aux2_d[:])
            cc = aux[:, CC_OFF:CC_OFF + 6]
            u345t = aux[0:RANK, U345_OFF:U345_OFF + 24]

            # ---- PE warm-up on zero data from early startup: shared lhsT,
            # results discarded. Keeps the tensor engine continuously busy
            # through the index chain so real matmuls run at ramped clock.
            warm = cpool.tile([KV, HALF], BF16)
            nc.gpsimd.memset(warm[:], 0.0)
            pd = dpool.tile([MV, HALF], F32)
            for _ in range(N_WARM):
                nc.tensor.matmul(
                    pd[:], lhsT=warm[:, 0:MV], rhs=warm[:], start=True,
                    stop=True,
                )

            # ---- block-diag [U0;U1;U2] bf16 cast on GpSimd
            ublk = cpool.tile([KV, MV], BF16)
            nc.gpsimd.tensor_copy(out=ublk[:], in_=aux2[:])

            # ---- W[r, e] = U3[d,r] * U4[e2,r] * U5[f,r], e = 64d + 8e2 + f
            # on DVE before the index broadcast lands (aux is tiny).
            t45 = cpool.tile([RANK, 64], F32)
            nc.vector.tensor_tensor(
                out=t45[:].rearrange("r (e f) -> r e f", e=8),
                in0=u345t[:, 8:16].unsqueeze(2).broadcast_to([RANK, 8, 8]),
                in1=u345t[:, 16:24].unsqueeze(1).broadcast_to([RANK, 8, 8]),
                op=ALU.mult,
            )
            wt = cpool.tile([RANK, EMB], F32R)
            nc.vector.tensor_tensor(
                out=wt[:].rearrange("r (d ef) -> r d ef", d=8),
                in0=u345t[:, 0:8].unsqueeze(2).broadcast_to([RANK, 8, 64]),
                in1=t45[:].unsqueeze(1).broadcast_to([RANK, 8, 64]),
                op=ALU.mult,
            )

            # ---- 4-op digit-decomposition chain, int16 2x DVE perf mode;
            # the final is_equal is split per half for an earlier gather.
            s1 = cpool.tile([KV, PER_CORE], I16)
            nc.vector.tensor_scalar(
                out=s1[:], in0=xrep[:], scalar1=cc[:, 0:1], scalar2=cc[:, 1:2],
                op0=ALU.add, op1=ALU.mult,
            )
            s2 = cpool.tile([KV, PER_CORE], I16)
            nc.vector.tensor_scalar(
                out=s2[:], in0=xrep[:], scalar1=cc[:, 2:3], scalar2=cc[:, 3:4],
                op0=ALU.add, op1=ALU.mult,
            )
            tkp = cpool.tile([KV, PER_CORE], I16)
            nc.vector.tensor_scalar(
                out=tkp[:], in0=s2[:], scalar1=cc[:, 4:5], scalar2=cc[:, 5:6],
                op0=ALU.mult, op1=ALU.subtract,
            )
            onehot = cpool.tile([KV, PER_CORE], BF16)
            for h in range(2):
                hs = slice(h * HALF, (h + 1) * HALF)
                nc.vector.tensor_tensor(
                    out=onehot[:, hs], in0=s1[:, hs], in1=tkp[:, hs],
                    op=ALU.is_equal,
                )

            # ---- gathers, then PE gap fillers (real rhs so they schedule
            # after the chain; results discarded) to hold the p-state while
            # the first V pieces are produced.
            pvs = []
            for h in range(2):
                pv = ppool.tile([MV, HALF], F32, name=f"pv_{h}", tag="pv")
                nc.tensor.matmul(
                    pv[:], lhsT=ublk[:],
                    rhs=onehot[:, h * HALF:(h + 1) * HALF],
                    start=True, stop=True,
                )
                pvs.append(pv)
            for _ in range(N_FILL):
                nc.tensor.matmul(
                    pd[:], lhsT=ublk[:], rhs=onehot[:, 0:HALF], start=True,
                    stop=True,
                )

            QC = HALF // 2  # 256-col product pieces
            for h in range(2):
                pv = pvs[h]
                # V = pv[0:32] * pv[32:64] * pv[64:96], in 256-col pieces.
                # DVE may read only one PSUM operand per op: stage block 0
                # to SBUF on the Scalar engine first.
                vth = cpool.tile([RANK, HALF], F32R, name=f"vt_{h}")
                for q in range(2):
                    qs = slice(q * QC, (q + 1) * QC)
                    s0 = wpool.tile([RANK, QC], F32, name=f"s0_{h}{q}",
                                    tag="s0")
                    nc.scalar.copy(out=s0[:], in_=pv[0:32, qs])
                    v01 = wpool.tile([RANK, QC], F32, name=f"v01_{h}{q}",
                                     tag="v01")
                    nc.vector.tensor_tensor(
                        out=v01[:], in0=s0[:], in1=pv[32:64, qs], op=ALU.mult
                    )
                    nc.vector.tensor_tensor(
                        out=vth[:, qs], in0=v01[:], in1=pv[64:96, qs],
                        op=ALU.mult,
                    )

                for g in range(2):
                    po2 = oppool.tile([128, 2 * EMB], F32, name=f"po_{h}{g}",
                                      tag="po")
                    for j in range(2):
                        nc.tensor.matmul(
                            po2[:, j * EMB:(j + 1) * EMB],
                            lhsT=vth[:, (2 * g + j) * 128:(2 * g + j + 1) * 128],
                            rhs=wt[:],
                            start=True, stop=True,
                        )
                    # bf16 staging copy in two 512-col halves, Scalar||Vector
                    osb = opool.tile([128, 2 * EMB], BF16, name=f"osb_{h}{g}",
                                     tag="osb")
                    nc.scalar.copy(out=osb[:, 0:EMB], in_=po2[:, 0:EMB])
                    nc.vector.tensor_copy(
                        out=osb[:, EMB:2 * EMB], in_=po2[:, EMB:2 * EMB]
                    )

                    for (ph, pg, pj, lo, hi, ring) in _OUT_PIECES:
                        if ph != h or pg != g:
                            continue
                        r0 = h * HALF + g * 256 + pj * 128
                        eng = nc.sync if ring == "sync" else nc.scalar
                        eng.dma_start(
                            out=out[r0 + lo:r0 + hi, :],
                            in_=osb[lo:hi, pj * EMB:(pj + 1) * EMB],
                        )

    nc.compile()
    return nc


_CACHE: dict = {}


def _get_nc():
    if "nc" not in _CACHE:
        _CACHE["nc"] = build()
    return _CACHE["nc"]


def run(inputs, **spmd_kwargs):
    nc = _get_nc()
    x = np.ascontiguousarray(inputs["x"].reshape(-1)).astype(np.uint16)
    us = [
        np.ascontiguousarray(inputs[f"U{j}"], dtype=np.float32) for j in range(6)
    ]
    aux, aux2 = _aux_table(us)
    in_maps = []
    for i in range(N_CORES):
        in_maps.append(
            {"x": x[i * PER_CORE:(i + 1) * PER_CORE], "aux": aux, "aux2": aux2}
        )
    res = run_bass_kernel_spmd(
        nc, in_maps, core_ids=list(range(N_CORES)), **spmd_kwargs
    )
    shards = [
        np.asarray(res.results[i]["out"]).astype(np.float32)
        for i in range(N_CORES)
    ]
    full = np.concatenate(shards, axis=0).reshape(4, 2048, EMB)
    return full, res


def kernel(**inputs) -> np.ndarray:
    return run(inputs)[0]
